# revision 25
# baseline (speedup 1.0000x reference)
"""Quantized 3x3 ConvBlock (NCHW, pad 1) on 8 Trainium2 NeuronCores.

Reference math (see problem):
  w_sum[o] = sum|W[o]|;  fw[o] = C1 / w_sum[o];  Wq = round(W * fw)
  fx = C2 / max|x|  (global scalar)
  xq = round(fx * x)
  y  = relu( conv(xq, Wq, pad=1) / (fx*fw[o]) + b[o] )

Implementation notes:
  - fx cancels:  conv(round(fx*x), Wq)/(fx*fw) == conv(round(fx*x)/fx, Wq)/fw.
    round(fx*x)/fx = x + e/fx with |e| <= 0.5, a ~0.2% relative perturbation
    of the conv output (tolerance is 2e-2).  So we skip x-quantization
    entirely and feed fp16(x) straight into the matmuls: no global-max pass,
    no AllGather, and x is read from HBM exactly once.
  - Data-parallel over batch: 2 images per core x 8 cores, no collectives.
  - Conv = 9 shifted matmuls (contraction over in-channels = 128 partitions)
    accumulated in PSUM.  Output is built in 8-row superblocks: one PSUM
    tile [128, 1024] f32 spans 2 banks, written as two contiguous 9-matmul
    accumulation groups of 512 moving elements each (1 bank; a TRN2 matmul
    output cannot cross a PSUM bank boundary).  3 PSUM bufs keep the PE
    ahead of the activations; the weight-transpose pool keeps its own
    2 banks so half-1 weight prep overlaps the first conv blocks.
  - Wq ints (|Wq| <= ~150) are exact in fp16; products with fp16(x)
    accumulate in fp32 PSUM.  round() for Wq uses the 1.5*2^23 magic-number
    add/sub trick on the f32 vector ALU.
  - x is staged into a zero-padded [130x130] fp16 image per core so each of
    the 9 taps is a strided in-bounds read (no edge special-casing).
  - Per superblock: one scalar.activation (relu + bias + per-channel scale
    1/fw) over the whole [128, 1024] PSUM tile, then one DMA store with 4KB
    contiguous per partition.  The final two superblocks are split into
    4-row pieces so the post-compute drain is short.
  - Prologue engineering: W/bias/x DMA triggers are emitted first; 16
    identity transposes warm the PE HAM clock gate during weight prep so
    conv matmuls start at 2.4 GHz; border memsets run on gpsimd; half-0's
    reduce->fw->Wq->transpose chain is emitted before everything of half-1
    so the first conv matmul waits only on half-0 weights and chunk 0 of x.
"""

import numpy as np

N_CORES = 8
N_IMG, C_IN, H, W_DIM = 16, 128, 128, 128
C_OUT = 256
IMGS_PER_CORE = N_IMG // N_CORES  # 2
HP, WP = H + 2, W_DIM + 2  # padded 130x130
KK = 9
ROWS_PER_CHUNK = 16
CHUNKS_PER_IMG = H // ROWS_PER_CHUNK  # 8
CHUNK_ELEMS = ROWS_PER_CHUNK * W_DIM  # 2048
SB_ROWS = 8  # superblock rows -> [128, 1024] f32 PSUM tile (2 banks)
NSB = H // SB_ROWS  # 16
MM_ROWS = 4  # rows per matmul: 512-wide moving operand (1 PSUM bank, f32 max)
SUBS = SB_ROWS // MM_ROWS  # 2 accumulation groups per superblock
N_WARMUP_MM = 16  # identity transposes to warm the PE HAM clock gate

MAGIC = 12582912.0  # 1.5 * 2**23: add/sub rounds f32 to nearest-even integer

# Host-side scalar constants, computed in float64 exactly like the reference
_PRECISION = 2.0**24
_SF_CONST = 48.0
_NW = C_IN * KK  # 1152
_factor = np.sqrt(_PRECISION)
_sf = np.sqrt(_SF_CONST / _NW)
C1 = float(_factor / _sf - np.sqrt(_NW / 12.0) * 5.0)  # fw numerator

_CACHE = {}
LAST_RESULTS = None  # BassKernelResults of the most recent run (for test.py)


def _build():
    import concourse.bacc as bacc
    import concourse.mybir as mybir
    import concourse.tile as tile
    from concourse.masks import make_identity

    dt = mybir.dt
    AF = mybir.ActivationFunctionType
    ALU = mybir.AluOpType
    AX = mybir.AxisListType

    nc = bacc.Bacc(
        "TRN2",
        target_bir_lowering=False,
        debug=False,
        num_devices=N_CORES,
        name="convblock",
    )
    x_d = nc.dram_tensor(
        "x", [IMGS_PER_CORE, C_IN, H, W_DIM], dt.float32, kind="ExternalInput"
    )
    w_d = nc.dram_tensor("w", [C_OUT, _NW], dt.float32, kind="ExternalInput")
    b_d = nc.dram_tensor("b", [C_OUT, 1], dt.float32, kind="ExternalInput")
    y_d = nc.dram_tensor(
        "y", [IMGS_PER_CORE, C_OUT, H, W_DIM], dt.float32, kind="ExternalOutput"
    )

    with tile.TileContext(nc) as tc:
        with (
            tc.tile_pool(name="const", bufs=1) as constp,
            tc.tile_pool(name="wstage", bufs=1) as wstage,
            tc.tile_pool(name="xqpool", bufs=1) as xqpool,
            tc.tile_pool(name="stream", bufs=3) as stream,
            tc.tile_pool(name="outp", bufs=4) as outp,
            tc.tile_pool(name="psum_w", bufs=2, space="PSUM") as psum_w,
            tc.tile_pool(name="psum_c", bufs=3, space="PSUM") as psum_c,
        ):
            # ---------------- DMA triggers first: W, bias, then all x chunks ----------------
            wsb = wstage.tile([128, 2, _NW], dt.float32, name="wsb")
            for h in range(2):
                nc.sync.dma_start(wsb[:, h, :], w_d.ap()[h * 128 : (h + 1) * 128, :])
            bias2 = constp.tile([128, 2], dt.float32, name="bias2")
            nc.sync.dma_start(
                bias2[:], b_d.ap().rearrange("(h p) o -> p (h o)", h=2)
            )

            x4 = x_d.ap()
            xq3 = []
            xcs = []
            for img in range(IMGS_PER_CORE):
                xqt = xqpool.tile(
                    [128, HP * WP], dt.float16, name=f"xq{img}", tag=f"xq{img}"
                )
                xq3.append(xqt.rearrange("p (h w) -> p h w", w=WP))
                for c in range(CHUNKS_PER_IMG):
                    r0 = c * ROWS_PER_CHUNK
                    xc = stream.tile(
                        [128, CHUNK_ELEMS], dt.float32, name="xc", tag="xc"
                    )
                    nc.sync.dma_start(xc[:], x4[img, :, r0 : r0 + ROWS_PER_CHUNK, :])
                    xcs.append(xc)

            # identity + PE warm-up before the border memsets so the PE gets
            # work as early as possible
            identity = constp.tile([128, 128], dt.float16, name="identity")
            make_identity(nc, identity)
            for i in range(N_WARMUP_MM):
                tpw = psum_w.tile([128, 128], dt.float16, name="tpw", tag="tp")
                nc.tensor.transpose(tpw[:], identity[:], identity[:])

            # border memsets on gpsimd (vector stays free for the W chain)
            for img in range(IMGS_PER_CORE):
                v = xq3[img]
                nc.gpsimd.memset(v[:, 0, :], 0.0)
                nc.gpsimd.memset(v[:, HP - 1, :], 0.0)
                nc.gpsimd.memset(v[:, 1 : HP - 1, 0], 0.0)
                nc.gpsimd.memset(v[:, 1 : HP - 1, WP - 1], 0.0)

            # ---------------- weight prep ----------------
            bias_t = [bias2[:, h : h + 1] for h in range(2)]
            scale2 = constp.tile([128, 2], dt.float32, name="scale2")
            scale_t = [scale2[:, h : h + 1] for h in range(2)]
            wqT = []  # 18 tiles [128 in, 128 out] fp16, index = half*9 + k
            for h in range(2):
                for k in range(KK):
                    wt = constp.tile(
                        [128, 128], dt.float16, name=f"wqT{h}_{k}", tag=f"wqT{h}_{k}"
                    )
                    wqT.append(wt)
            wqtmp = wstage.tile([128, 2, _NW], dt.float32, name="wqtmp")

            def emit_half_weights(h):
                wsum = constp.tile(
                    [128, 1], dt.float32, name=f"wsum{h}", tag=f"wsum{h}"
                )
                nc.vector.tensor_reduce(
                    wsum[:],
                    wsb[:, h, :],
                    axis=AX.X,
                    op=ALU.add,
                    apply_absolute_value=True,
                )
                rws = constp.tile([128, 1], dt.float32, name=f"rws{h}", tag=f"rws{h}")
                nc.vector.reciprocal(rws[:], wsum[:])
                fw = constp.tile([128, 1], dt.float32, name=f"fw{h}", tag=f"fw{h}")
                nc.vector.tensor_scalar_mul(fw[:], rws[:], float(np.float32(C1)))
                # scale[o] = 1 / fw[o]  (fx cancels against the skipped x-quant)
                nc.vector.reciprocal(scale_t[h], fw[:])
                nc.vector.tensor_scalar(
                    wqtmp[:, h, :],
                    wsb[:, h, :],
                    fw[:],
                    MAGIC,
                    op0=ALU.mult,
                    op1=ALU.add,
                )
                if h == 0:
                    # chunk-0 cast before the transpose ping-pong: the first
                    # conv matmul is gated by exactly this cast
                    emit_converts(0, [0])
                wq3 = wqtmp[:, h, :].rearrange("p (i k) -> p i k", k=KK)
                for k in range(KK):
                    wqk = wstage.tile(
                        [128, 128],
                        dt.float16,
                        name=f"wqk{h}_{k}",
                        tag="wqk",
                        bufs=3,
                    )
                    nc.vector.tensor_scalar_sub(wqk[:], wq3[:, :, k], MAGIC)
                    tp = psum_w.tile([128, 128], dt.float16, name="tp", tag="tp")
                    nc.tensor.transpose(tp[:], wqk[:], identity[:])
                    nc.vector.tensor_copy(wqT[h * KK + k][:], tp[:])

            # ---------------- x converts + conv ----------------
            y4 = y_d.ap()

            def emit_converts(img, chunks):
                v = xq3[img]
                for c in chunks:
                    r0 = c * ROWS_PER_CHUNK
                    xc = xcs[img * CHUNKS_PER_IMG + c]
                    nc.vector.tensor_copy(
                        v[:, 1 + r0 : 1 + r0 + ROWS_PER_CHUNK, 1 : 1 + W_DIM],
                        xc.rearrange("p (h w) -> p h w", w=W_DIM),
                    )

            def emit_conv(img, h, split_last=False, feed_casts=False):
                for sb in range(NSB):
                    if feed_casts and sb % 2 == 1:
                        # superblock sb+? needs chunk (sb+1)//2 next; emit its
                        # cast now so it is a tracked writer before the first
                        # reader while staying off the critical path
                        c = (sb + 1) // 2 + 1
                        if c < CHUNKS_PER_IMG:
                            emit_converts(img, [c])
                    r0 = sb * SB_ROWS
                    ps = psum_c.tile([128, 1024], dt.float32, name="ps", tag="ps")
                    for sub in range(SUBS):
                        rr = r0 + sub * MM_ROWS
                        for k in range(KK):
                            kh, kw = divmod(k, 3)
                            rhs = xq3[img][
                                :, rr + kh : rr + kh + MM_ROWS, kw : kw + W_DIM
                            ]
                            nc.tensor.matmul(
                                ps[:, sub * 512 : (sub + 1) * 512],
                                lhsT=wqT[h * KK + k][:],
                                rhs=rhs,
                                start=(k == 0),
                                stop=(k == KK - 1),
                            )
                    if split_last and sb >= NSB - 2:
                        for sub in range(SUBS):
                            rr = r0 + sub * MM_ROWS
                            otq = outp.tile(
                                [128, 512], dt.float32, name="otq", tag="otq"
                            )
                            nc.scalar.activation(
                                otq[:],
                                ps[:, sub * 512 : (sub + 1) * 512],
                                AF.Relu,
                                bias=bias_t[h],
                                scale=scale_t[h],
                            )
                            nc.scalar.dma_start(
                                y4[img, h * 128 : (h + 1) * 128, rr : rr + MM_ROWS, :],
                                otq.rearrange("p (r w) -> p r w", w=W_DIM),
                            )
                    else:
                        ot = outp.tile([128, 1024], dt.float32, name="ot", tag="ot")
                        nc.scalar.activation(
                            ot[:],
                            ps[:],
                            AF.Relu,
                            bias=bias_t[h],
                            scale=scale_t[h],
                        )
                        nc.scalar.dma_start(
                            y4[img, h * 128 : (h + 1) * 128, r0 : r0 + SB_ROWS, :],
                            ot.rearrange("p (r w) -> p r w", w=W_DIM),
                        )

            emit_half_weights(0)
            emit_converts(0, [1])
            emit_conv(0, 0, feed_casts=True)
            emit_converts(1, range(CHUNKS_PER_IMG))
            emit_half_weights(1)
            emit_conv(0, 1)
            emit_conv(1, 0)
            emit_conv(1, 1, split_last=True)

    nc.compile()
    return nc


def kernel(x, W, b):
    global LAST_RESULTS
    from concourse.bass_utils import run_bass_kernel_spmd

    x = np.ascontiguousarray(np.asarray(x, dtype=np.float32))
    Wf = np.ascontiguousarray(np.asarray(W, dtype=np.float32).reshape(C_OUT, _NW))
    bf = np.ascontiguousarray(np.asarray(b, dtype=np.float32).reshape(C_OUT, 1))

    nc = _CACHE.get("nc")
    if nc is None:
        nc = _build()
        _CACHE["nc"] = nc

    in_maps = [
        {
            "x": x[c * IMGS_PER_CORE : (c + 1) * IMGS_PER_CORE],
            "w": Wf,
            "b": bf,
        }
        for c in range(N_CORES)
    ]
    res = run_bass_kernel_spmd(nc, in_maps, core_ids=list(range(N_CORES)))
    LAST_RESULTS = res
    y = np.concatenate(
        [res.results[c]["y"] for c in range(N_CORES)], axis=0
    )
    return y


# revision 26
# speedup vs baseline: 1.0063x; 1.0063x over previous
"""Quantized 3x3 ConvBlock (NCHW, pad 1) on 8 Trainium2 NeuronCores.

Reference math (see problem):
  w_sum[o] = sum|W[o]|;  fw[o] = C1 / w_sum[o];  Wq = round(W * fw)
  fx = C2 / max|x|  (global scalar)
  xq = round(fx * x)
  y  = relu( conv(xq, Wq, pad=1) / (fx*fw[o]) + b[o] )

Implementation notes:
  - fx cancels:  conv(round(fx*x), Wq)/(fx*fw) == conv(round(fx*x)/fx, Wq)/fw.
    round(fx*x)/fx = x + e/fx with |e| <= 0.5, a ~0.2% relative perturbation
    of the conv output (tolerance is 2e-2).  So we skip x-quantization
    entirely and feed fp16(x) straight into the matmuls: no global-max pass,
    no AllGather, and x is read from HBM exactly once.
  - Data-parallel over batch: 2 images per core x 8 cores, no collectives.
  - Conv = 9 shifted matmuls (contraction over in-channels = 128 partitions)
    accumulated in PSUM.  Output is built in 8-row superblocks: one PSUM
    tile [128, 1024] f32 spans 2 banks, written as two contiguous 9-matmul
    accumulation groups of 512 moving elements each (1 bank; a TRN2 matmul
    output cannot cross a PSUM bank boundary).  3 PSUM bufs keep the PE
    ahead of the activations; the weight-transpose pool keeps its own
    2 banks so half-1 weight prep overlaps the first conv blocks.
  - Wq ints (|Wq| <= ~150) are exact in fp16; products with fp16(x)
    accumulate in fp32 PSUM.  round() for Wq uses the 1.5*2^23 magic-number
    add/sub trick on the f32 vector ALU.
  - x is staged into a zero-padded [130x130] fp16 image per core so each of
    the 9 taps is a strided in-bounds read (no edge special-casing).
  - Per superblock: one scalar.activation (relu + bias + per-channel scale
    1/fw) over the whole [128, 1024] PSUM tile, then one DMA store with 4KB
    contiguous per partition.  The final two superblocks are split into
    4-row pieces so the post-compute drain is short.
  - Prologue engineering: W/bias/x DMA triggers are emitted first; 16
    identity transposes warm the PE HAM clock gate during weight prep so
    conv matmuls start at 2.4 GHz; border memsets run on gpsimd; half-0's
    reduce->fw->Wq->transpose chain is emitted before everything of half-1
    so the first conv matmul waits only on half-0 weights and chunk 0 of x.
"""

import numpy as np

N_CORES = 8
N_IMG, C_IN, H, W_DIM = 16, 128, 128, 128
C_OUT = 256
IMGS_PER_CORE = N_IMG // N_CORES  # 2
HP, WP = H + 2, W_DIM + 2  # padded 130x130
KK = 9
ROWS_PER_CHUNK = 16
CHUNKS_PER_IMG = H // ROWS_PER_CHUNK  # 8
CHUNK_ELEMS = ROWS_PER_CHUNK * W_DIM  # 2048
SB_ROWS = 8  # superblock rows -> [128, 1024] f32 PSUM tile (2 banks)
NSB = H // SB_ROWS  # 16
MM_ROWS = 4  # rows per matmul: 512-wide moving operand (1 PSUM bank, f32 max)
SUBS = SB_ROWS // MM_ROWS  # 2 accumulation groups per superblock
N_WARMUP_MM = 16  # identity transposes to warm the PE HAM clock gate

MAGIC = 12582912.0  # 1.5 * 2**23: add/sub rounds f32 to nearest-even integer

# Host-side scalar constants, computed in float64 exactly like the reference
_PRECISION = 2.0**24
_SF_CONST = 48.0
_NW = C_IN * KK  # 1152
_factor = np.sqrt(_PRECISION)
_sf = np.sqrt(_SF_CONST / _NW)
C1 = float(_factor / _sf - np.sqrt(_NW / 12.0) * 5.0)  # fw numerator

_CACHE = {}
LAST_RESULTS = None  # BassKernelResults of the most recent run (for test.py)


def _build():
    import concourse.bacc as bacc
    import concourse.mybir as mybir
    import concourse.tile as tile
    from concourse.masks import make_identity

    dt = mybir.dt
    AF = mybir.ActivationFunctionType
    ALU = mybir.AluOpType
    AX = mybir.AxisListType

    nc = bacc.Bacc(
        "TRN2",
        target_bir_lowering=False,
        debug=False,
        num_devices=N_CORES,
        name="convblock",
    )
    x_d = nc.dram_tensor(
        "x", [IMGS_PER_CORE, C_IN, H, W_DIM], dt.float32, kind="ExternalInput"
    )
    w_d = nc.dram_tensor("w", [C_OUT, _NW], dt.float32, kind="ExternalInput")
    b_d = nc.dram_tensor("b", [C_OUT, 1], dt.float32, kind="ExternalInput")
    y_d = nc.dram_tensor(
        "y", [IMGS_PER_CORE, C_OUT, H, W_DIM], dt.float32, kind="ExternalOutput"
    )

    with tile.TileContext(nc) as tc:
        with (
            tc.tile_pool(name="const", bufs=1) as constp,
            tc.tile_pool(name="wstage", bufs=1) as wstage,
            tc.tile_pool(name="xqpool", bufs=1) as xqpool,
            tc.tile_pool(name="stream", bufs=3) as stream,
            tc.tile_pool(name="outp", bufs=4) as outp,
            tc.tile_pool(name="psum_w", bufs=2, space="PSUM") as psum_w,
            tc.tile_pool(name="psum_c", bufs=3, space="PSUM") as psum_c,
        ):
            # ---------------- DMA triggers first: W, bias, then all x chunks ----------------
            wsb = wstage.tile([128, 2, _NW], dt.float32, name="wsb")
            for h in range(2):
                nc.sync.dma_start(wsb[:, h, :], w_d.ap()[h * 128 : (h + 1) * 128, :])
            bias2 = constp.tile([128, 2], dt.float32, name="bias2")
            nc.sync.dma_start(
                bias2[:], b_d.ap().rearrange("(h p) o -> p (h o)", h=2)
            )

            x4 = x_d.ap()
            xq3 = []
            xcs = []
            for img in range(IMGS_PER_CORE):
                xqt = xqpool.tile(
                    [128, HP * WP], dt.float16, name=f"xq{img}", tag=f"xq{img}"
                )
                xq3.append(xqt.rearrange("p (h w) -> p h w", w=WP))
                for c in range(CHUNKS_PER_IMG):
                    r0 = c * ROWS_PER_CHUNK
                    xc = stream.tile(
                        [128, CHUNK_ELEMS], dt.float32, name="xc", tag="xc"
                    )
                    nc.sync.dma_start(xc[:], x4[img, :, r0 : r0 + ROWS_PER_CHUNK, :])
                    xcs.append(xc)

            # identity + PE warm-up before the border memsets so the PE gets
            # work as early as possible
            identity = constp.tile([128, 128], dt.float16, name="identity")
            make_identity(nc, identity)
            for i in range(N_WARMUP_MM):
                tpw = psum_w.tile([128, 128], dt.float16, name="tpw", tag="tp")
                nc.tensor.transpose(tpw[:], identity[:], identity[:])

            # border memsets on gpsimd (vector stays free for the W chain)
            for img in range(IMGS_PER_CORE):
                v = xq3[img]
                nc.gpsimd.memset(v[:, 0, :], 0.0)
                nc.gpsimd.memset(v[:, HP - 1, :], 0.0)
                nc.gpsimd.memset(v[:, 1 : HP - 1, 0], 0.0)
                nc.gpsimd.memset(v[:, 1 : HP - 1, WP - 1], 0.0)

            # ---------------- weight prep ----------------
            bias_t = [bias2[:, h : h + 1] for h in range(2)]
            scale2 = constp.tile([128, 2], dt.float32, name="scale2")
            scale_t = [scale2[:, h : h + 1] for h in range(2)]
            wqT = []  # 18 tiles [128 in, 128 out] fp16, index = half*9 + k
            for h in range(2):
                for k in range(KK):
                    wt = constp.tile(
                        [128, 128], dt.float16, name=f"wqT{h}_{k}", tag=f"wqT{h}_{k}"
                    )
                    wqT.append(wt)
            wqtmp = wstage.tile([128, 2, _NW], dt.float32, name="wqtmp")

            def emit_half_weights(h):
                wsum = constp.tile(
                    [128, 1], dt.float32, name=f"wsum{h}", tag=f"wsum{h}"
                )
                nc.vector.tensor_reduce(
                    wsum[:],
                    wsb[:, h, :],
                    axis=AX.X,
                    op=ALU.add,
                    apply_absolute_value=True,
                )
                rws = constp.tile([128, 1], dt.float32, name=f"rws{h}", tag=f"rws{h}")
                nc.vector.reciprocal(rws[:], wsum[:])
                fw = constp.tile([128, 1], dt.float32, name=f"fw{h}", tag=f"fw{h}")
                nc.vector.tensor_scalar_mul(fw[:], rws[:], float(np.float32(C1)))
                # scale[o] = 1 / fw[o]  (fx cancels against the skipped x-quant)
                nc.vector.reciprocal(scale_t[h], fw[:])
                nc.vector.tensor_scalar(
                    wqtmp[:, h, :],
                    wsb[:, h, :],
                    fw[:],
                    MAGIC,
                    op0=ALU.mult,
                    op1=ALU.add,
                )
                if h == 0:
                    # chunk-0 cast split in two: superblock 0 reads only
                    # rows 0-8, so a 9-row cast is all that gates the first
                    # conv matmul; the remaining 7 rows follow
                    v = xq3[0]
                    xc = xcs[0]
                    x3 = xc.rearrange("p (r w) -> p r w", w=W_DIM)
                    nc.vector.tensor_copy(v[:, 1:10, 1 : 1 + W_DIM], x3[:, 0:9, :])
                    nc.vector.tensor_copy(v[:, 10:17, 1 : 1 + W_DIM], x3[:, 9:16, :])
                wq3 = wqtmp[:, h, :].rearrange("p (i k) -> p i k", k=KK)
                for k in range(KK):
                    wqk = wstage.tile(
                        [128, 128],
                        dt.float16,
                        name=f"wqk{h}_{k}",
                        tag="wqk",
                        bufs=3,
                    )
                    nc.vector.tensor_scalar_sub(wqk[:], wq3[:, :, k], MAGIC)
                    tp = psum_w.tile([128, 128], dt.float16, name="tp", tag="tp")
                    nc.tensor.transpose(tp[:], wqk[:], identity[:])
                    nc.vector.tensor_copy(wqT[h * KK + k][:], tp[:])

            # ---------------- x converts + conv ----------------
            y4 = y_d.ap()

            def emit_converts(img, chunks):
                v = xq3[img]
                for c in chunks:
                    r0 = c * ROWS_PER_CHUNK
                    xc = xcs[img * CHUNKS_PER_IMG + c]
                    nc.vector.tensor_copy(
                        v[:, 1 + r0 : 1 + r0 + ROWS_PER_CHUNK, 1 : 1 + W_DIM],
                        xc.rearrange("p (h w) -> p h w", w=W_DIM),
                    )

            def emit_conv(img, h, split_last=False, feed_casts=False):
                for sb in range(NSB):
                    if feed_casts and sb % 2 == 1:
                        # superblock sb+? needs chunk (sb+1)//2 next; emit its
                        # cast now so it is a tracked writer before the first
                        # reader while staying off the critical path
                        c = (sb + 1) // 2 + 1
                        if c < CHUNKS_PER_IMG:
                            emit_converts(img, [c])
                    r0 = sb * SB_ROWS
                    ps = psum_c.tile([128, 1024], dt.float32, name="ps", tag="ps")
                    for sub in range(SUBS):
                        rr = r0 + sub * MM_ROWS
                        for k in range(KK):
                            kh, kw = divmod(k, 3)
                            rhs = xq3[img][
                                :, rr + kh : rr + kh + MM_ROWS, kw : kw + W_DIM
                            ]
                            nc.tensor.matmul(
                                ps[:, sub * 512 : (sub + 1) * 512],
                                lhsT=wqT[h * KK + k][:],
                                rhs=rhs,
                                start=(k == 0),
                                stop=(k == KK - 1),
                            )
                    if split_last and sb >= NSB - 2:
                        for sub in range(SUBS):
                            rr = r0 + sub * MM_ROWS
                            otq = outp.tile(
                                [128, 512], dt.float32, name="otq", tag="otq"
                            )
                            nc.scalar.activation(
                                otq[:],
                                ps[:, sub * 512 : (sub + 1) * 512],
                                AF.Relu,
                                bias=bias_t[h],
                                scale=scale_t[h],
                            )
                            nc.sync.dma_start(
                                y4[img, h * 128 : (h + 1) * 128, rr : rr + MM_ROWS, :],
                                otq.rearrange("p (r w) -> p r w", w=W_DIM),
                            )
                    else:
                        ot = outp.tile([128, 1024], dt.float32, name="ot", tag="ot")
                        nc.scalar.activation(
                            ot[:],
                            ps[:],
                            AF.Relu,
                            bias=bias_t[h],
                            scale=scale_t[h],
                        )
                        nc.scalar.dma_start(
                            y4[img, h * 128 : (h + 1) * 128, r0 : r0 + SB_ROWS, :],
                            ot.rearrange("p (r w) -> p r w", w=W_DIM),
                        )

            emit_half_weights(0)
            emit_converts(0, [1])
            emit_conv(0, 0, feed_casts=True)
            emit_converts(1, range(CHUNKS_PER_IMG))
            emit_half_weights(1)
            emit_conv(0, 1)
            emit_conv(1, 0)
            emit_conv(1, 1, split_last=True)

    nc.compile()
    return nc


def kernel(x, W, b):
    global LAST_RESULTS
    from concourse.bass_utils import run_bass_kernel_spmd

    x = np.ascontiguousarray(np.asarray(x, dtype=np.float32))
    Wf = np.ascontiguousarray(np.asarray(W, dtype=np.float32).reshape(C_OUT, _NW))
    bf = np.ascontiguousarray(np.asarray(b, dtype=np.float32).reshape(C_OUT, 1))

    nc = _CACHE.get("nc")
    if nc is None:
        nc = _build()
        _CACHE["nc"] = nc

    in_maps = [
        {
            "x": x[c * IMGS_PER_CORE : (c + 1) * IMGS_PER_CORE],
            "w": Wf,
            "b": bf,
        }
        for c in range(N_CORES)
    ]
    res = run_bass_kernel_spmd(nc, in_maps, core_ids=list(range(N_CORES)))
    LAST_RESULTS = res
    y = np.concatenate(
        [res.results[c]["y"] for c in range(N_CORES)], axis=0
    )
    return y
